# revision 6
# baseline (speedup 1.0000x reference)
"""Trainium2 Bass kernel for nn_BrainBottleneckLocal (dense_cnn).

Sharding: spatial rows. H=16 rows split 2-per-core across 8 NeuronCores; every
layer is core-local (the locally-connected layer needs a 1-row halo, provided
by recomputing conv1 on 4 rows; out-of-bounds taps are zeroed in the LC weight).

Per-core pipeline (all activations layout: partition=channel-chunk, free=(h,w,n)):
  conv1x1+BN1+ReLU (bf16, halo rows fp8e3) -> locally-connected 3x3 with fp8e3
  weights streamed as the moving operand, two locations computed concurrently on
  the two column halves of the PE array (tile_position col tiling), PE-transpose
  to channel-major, BN2+ReLU -> conv1x1+BN3+residual (bf16) -> opponent
  inhibition as a DoubleRow fp8e4 matmul (g and relu(t) quantized on chip)
  -> divide -> bf16 store (host upcasts to fp32).

DMA: big streams ride the sync HWDGE ring in FIFO order (xhalo, xmid, lcw
pairs) so conv1 inputs land before the LC weight stream; setup constants and
output stores ride the scalar ring.
"""

import math
from contextlib import ExitStack

import numpy as np

import concourse.bacc as bacc
import concourse.bass as bass
import concourse.mybir as mybir
import concourse.tile as tile
from concourse.bass_utils import run_bass_kernel_spmd

F32 = mybir.dt.float32
BF16 = mybir.dt.bfloat16
F8E3 = mybir.dt.float8e3
F8E4 = mybir.dt.float8e4
NPBF16 = mybir.dt.np(BF16)
NPF8E3 = mybir.dt.np(F8E3)

EPS = 1e-5
N, CIN, H, W = 64, 1024, 16, 16
WID, COUT = 256, 1024
NCORES = 8
RPC = H // NCORES          # rows per core = 2
HLO = RPC + 2              # rows incl halo = 4
WP = W + 2                 # padded width = 18
NLOC = RPC * W             # LC locations per core = 32
NPAIR = NLOC // 2          # 2-location groups = 16
KC = 18                    # LC contraction chunks: 9 taps x (256/128)
CC1 = CIN // 128           # 8
CCW = WID // 128           # 2
CC3 = COUT // 128          # 8
FR = RPC * W * N           # free size of per-core row block = 2048
ROWF = W * N               # 1024 free elems per (row)
PROWF = WP * N             # 1152 free elems per padded row

LCW_SCALE = 32.0           # lcw fp8e3 pre-scale
HALO_SCALE = 2.0           # xhalo fp8e3 pre-scale
G_SCALE = 16384.0          # g fp8e4 pre-scale (2^14)
RES_SCALE = 16.0           # relu(t) fp8e4 pre-scale
DEN_SCALE = 1.0 / (G_SCALE * RES_SCALE)

AF = mybir.ActivationFunctionType
ALU = mybir.AluOpType


def _declare_drams(nc):
    ap = {}
    ap["xhalo"] = nc.dram_tensor("xhalo", [CC1, 128, 2 * ROWF], F8E3,
                                 kind="ExternalInput").ap()
    ap["xmid"] = nc.dram_tensor("xmid", [CC1, 128, FR], BF16,
                                kind="ExternalInput").ap()
    ap["lcw"] = nc.dram_tensor("lcw", [NPAIR, 128, 2 * KC * WID], F8E3,
                               kind="ExternalInput").ap()
    ap["w1t"] = nc.dram_tensor("w1t", [CC1, 128, WID], BF16,
                               kind="ExternalInput").ap()
    ap["w3t"] = nc.dram_tensor("w3t", [CCW, 128, COUT], BF16,
                               kind="ExternalInput").ap()
    ap["b1"] = nc.dram_tensor("b1", [CCW, 128, 1], F32,
                              kind="ExternalInput").ap()
    ap["b2"] = nc.dram_tensor("b2", [CCW, 128, 1], F32,
                              kind="ExternalInput").ap()
    ap["i2s"] = nc.dram_tensor("i2s", [CCW, 128, 1], F32,
                               kind="ExternalInput").ap()
    ap["b3"] = nc.dram_tensor("b3", [CC3, 128, 1], F32,
                              kind="ExternalInput").ap()
    ap["sigs"] = nc.dram_tensor("sigs", [CC3, 128, 1], F32,
                                kind="ExternalInput").ap()
    ap["gt"] = nc.dram_tensor("gt", [128, 2048], BF16,
                              kind="ExternalInput").ap()
    ap["ident"] = nc.dram_tensor("ident", [128, 128], BF16,
                                 kind="ExternalInput").ap()
    # partition-major so the batched store keeps the SBUF partition dim
    # first: out[p, ns, cchunk, s] = channel cchunk*128+p, sample ns*512+s
    ap["out"] = nc.dram_tensor("out", [128, FR // 512, CC3, 512], BF16,
                               kind="ExternalOutput").ap()
    return ap


def _build_nc(ktimes: int = 1):
    nc = bacc.Bacc("TRN2", target_bir_lowering=False, debug=False,
                   num_devices=NCORES)
    ap = _declare_drams(nc)
    with tile.TileContext(nc) as tc:
        if ktimes == 1:
            _trace_kernel(tc, nc, ap)
        else:
            with tc.For_i(0, ktimes, 1):
                _trace_kernel(tc, nc, ap)
    nc.compile()
    return nc


def _trace_kernel(tc, nc, ap):
    with ExitStack() as ctx:
        persist = ctx.enter_context(tc.tile_pool(name="persist", bufs=1))
        xmid_pool = ctx.enter_context(tc.tile_pool(name="xmidp", bufs=1))
        out1p_pool = ctx.enter_context(tc.tile_pool(name="out1p", bufs=1))
        psum = ctx.enter_context(
            tc.tile_pool(name="psum", bufs=1, space="PSUM"))

        # ---- small constants (scalar ring) ------------------------------
        w1t_t = []
        for cc in range(CC1):
            t = persist.tile([128, WID], BF16, name=f"w1t_{cc}",
                             tag=f"w1t{cc}")
            nc.scalar.dma_start(out=t, in_=ap["w1t"][cc])
            w1t_t.append(t)
        w3t_t = []
        for oc in range(CCW):
            t = persist.tile([128, COUT], BF16, name=f"w3t_{oc}",
                             tag=f"w3t{oc}")
            nc.scalar.dma_start(out=t, in_=ap["w3t"][oc])
            w3t_t.append(t)
        ident_t = persist.tile([128, 128], BF16, name="ident", tag="ident")
        nc.scalar.dma_start(out=ident_t, in_=ap["ident"])
        gt_t = persist.tile([128, 2048], BF16, name="gt", tag="gt")
        nc.scalar.dma_start(out=gt_t, in_=ap["gt"])

        def load_bias(name, nch):
            outl = []
            for c in range(nch):
                t = persist.tile([128, 1], F32, name=f"{name}_{c}",
                                 tag=f"{name}{c}")
                nc.scalar.dma_start(out=t, in_=ap[name][c])
                outl.append(t)
            return outl

        b1_t = load_bias("b1", CCW)
        b2_t = load_bias("b2", CCW)
        i2s_t = load_bias("i2s", CCW)
        b3_t = load_bias("b3", CC3)
        sig_t = load_bias("sigs", CC3)

        # persistent activations
        out2_t = [persist.tile([128, NLOC * N], BF16, name=f"out2_{oc}",
                               tag=f"out2{oc}") for oc in range(CCW)]
        resb_t = [persist.tile([128, 2 * FR], F8E4, name=f"resb_{cp}",
                               tag=f"resb{cp}") for cp in range(CC3 // 2)]
        t_t = [persist.tile([128, FR], BF16, name=f"t_{oc}", tag=f"t{oc}")
               for oc in range(CC3)]
        gdr_t = [persist.tile([128, 2 * COUT], F8E4, name=f"gdr_{cp}",
                              tag=f"gdr{cp}") for cp in range(CC3 // 2)]

        # xmid persists to the residual add; xhalo only feeds conv1
        xmid_t = []
        for cc in range(CC1):
            t = xmid_pool.tile([128, FR], BF16, name=f"xm_{cc}",
                               tag=f"xm{cc}")
            xmid_t.append(t)

        # out1 padded: [128, (h4, w18, n64)], zeroed W-pad columns
        out1p_t = []
        for oc in range(CCW):
            t = out1p_pool.tile([128, HLO * PROWF], BF16,
                                name=f"out1p_{oc}", tag=f"out1p{oc}")
            nc.gpsimd.memset(t, 0.0)
            out1p_t.append(t)

        # ---- opponent-inhibition mixing matrix --------------------------
        # g8[c, o] = G_SCALE * exp(-d(o,c)^2/(2 s_c^2)) / Z_c, fp8e4, stored
        # as chunk-pairs [128, (cc2, o)] for the DoubleRow inhibition matmul.
        # gt[p, j] = bf16 d^2 template; chunk cc reads the 1024-wide slice at
        # j0 = 1024 - 128*cc (d^2 for c = 128cc+p vs all o).
        with ExitStack() as gctx:
            gtmp = gctx.enter_context(tc.tile_pool(name="gtmp", bufs=2))
            for cc in range(CC3):
                d2e = gtmp.tile([128, COUT], F32, name="d2e", tag="d2e")
                j0 = 1024 - 128 * cc
                nc.scalar.activation(out=d2e, in_=gt_t[:, j0:j0 + COUT],
                                     func=AF.Exp, scale=sig_t[cc], bias=0.0)
                esum = gtmp.tile([128, 1], F32, name="esum", tag="esum")
                nc.vector.reduce_sum(out=esum, in_=d2e,
                                     axis=mybir.AxisListType.X)
                nc.vector.reciprocal(out=esum, in_=esum)
                nc.vector.tensor_scalar(
                    out=gdr_t[cc // 2][:, (cc % 2) * COUT:(cc % 2 + 1) * COUT],
                    in0=d2e, scalar1=esum, scalar2=G_SCALE,
                    op0=ALU.mult, op1=ALU.mult)

            # ---- phase 1: conv1x1 #1 + BN1 + ReLU on 4 halo rows --------
            # sync-ring FIFO: xhalo first (rows -1,+2 fp8e3), then xmid.
            xh_pool = gctx.enter_context(tc.tile_pool(name="xhp", bufs=1))
            xh_t = []
            for cc in range(CC1):
                t = xh_pool.tile([128, 2 * ROWF], F8E3, name=f"xh_{cc}",
                                 tag=f"xh{cc}")
                nc.sync.dma_start(out=t, in_=ap["xhalo"][cc])
                xh_t.append(t)
            for cc in range(CC1):
                nc.sync.dma_start(out=xmid_t[cc], in_=ap["xmid"][cc])

            for h in range(HLO):
                halo = h in (0, HLO - 1)
                for oc in range(CCW):
                    for ns in range(2):
                        ps = psum.tile([128, 512], F32, name="ps1",
                                       tag="mm512", bufs=4)
                        for cc in range(CC1):
                            if halo:
                                hh = 0 if h == 0 else 1
                                mov = xh_t[cc][:, hh * ROWF + ns * 512:
                                               hh * ROWF + ns * 512 + 512]
                            else:
                                mov = xmid_t[cc][:, (h - 1) * ROWF + ns * 512:
                                                 (h - 1) * ROWF + ns * 512 + 512]
                            nc.tensor.matmul(
                                ps, w1t_t[cc][:, oc * 128:(oc + 1) * 128],
                                mov, start=(cc == 0), stop=(cc == CC1 - 1))
                        off = h * PROWF + N + ns * 512
                        nc.scalar.activation(
                            out=out1p_t[oc][:, off:off + 512], in_=ps,
                            func=AF.Relu, bias=b1_t[oc],
                            scale=(1.0 / HALO_SCALE) if halo else 1.0)

        # ---- phase 2: locally-connected 3x3 + BN2 + ReLU ---------------
        # Per pair of locations: 18 contraction chunks each, the two
        # locations' matmuls target the two column halves of the PE array
        # (tile_position) and run concurrently; psum [128(2x64n), 256(o)],
        # then 2 PE-transposes to [o, (loc, n)] and BN2 via per-partition
        # scale (inv2/LCW_SCALE) + bias.
        lcw_pool = ctx.enter_context(tc.tile_pool(name="lcwp", bufs=4))
        lct_pool = ctx.enter_context(tc.tile_pool(name="lctp", bufs=3))
        div_pool = ctx.enter_context(tc.tile_pool(name="divp", bufs=3))
        fin_pool = ctx.enter_context(tc.tile_pool(name="finp", bufs=2))

        for p in range(NPAIR):
            lw = lcw_pool.tile([128, 2 * KC * WID], F8E3, name="lcw_t",
                               tag="lcw")
            nc.sync.dma_start(out=lw, in_=ap["lcw"][p])
            ps2 = psum.tile([128, WID], F32, name="ps2", tag="lc", bufs=2)
            hlA, jA = divmod(2 * p, W)
            hlB, jB = divmod(2 * p + 1, W)
            for kc in range(KC):
                dk, ch = divmod(kc, 2)
                di, dj = divmod(dk, 3)
                offA = (hlA + di) * PROWF + (jA + dj) * N
                offB = (hlB + di) * PROWF + (jB + dj) * N
                nc.tensor.matmul(
                    ps2[0:64, :], out1p_t[ch][:, offA:offA + N],
                    lw[:, kc * WID:(kc + 1) * WID],
                    start=(kc == 0), stop=(kc == KC - 1),
                    tile_position=(0, 0), skip_group_check=True)
                nc.tensor.matmul(
                    ps2[64:128, :], out1p_t[ch][:, offB:offB + N],
                    lw[:, KC * WID + kc * WID:KC * WID + (kc + 1) * WID],
                    start=(kc == 0), stop=(kc == KC - 1),
                    tile_position=(0, 64), skip_group_check=True)
            tmpb = lct_pool.tile([128, WID], BF16, name="tmpb", tag="tmpb")
            nc.vector.tensor_copy(out=tmpb, in_=ps2)
            for oc in range(CCW):
                ptx = psum.tile([128, 128], BF16, name="ptx", tag="tp",
                                bufs=2)
                nc.tensor.transpose(ptx, tmpb[:, oc * 128:(oc + 1) * 128],
                                    ident_t)
                nc.scalar.activation(
                    out=out2_t[oc][:, p * 128:(p + 1) * 128], in_=ptx,
                    func=AF.Relu, bias=b2_t[oc], scale=i2s_t[oc])

        # ---- phase 3+4 per 512-slice of (hl,j,n): conv1x1 #2 + BN3 +
        # residual (bf16 xmid), then DoubleRow fp8 inhibition + divide +
        # bf16 store (batched per slice across all 8 channel chunks).
        for ns in range(FR // 512):
            sl = slice(ns * 512, ns * 512 + 512)
            fin = fin_pool.tile([128, CC3 * 512], BF16, name="fin",
                                tag="fin")
            for oc3 in range(CC3):
                ps = psum.tile([128, 512], F32, name="ps3", tag="mm512",
                               bufs=4)
                for oc in range(CCW):
                    nc.tensor.matmul(
                        ps, w3t_t[oc][:, oc3 * 128:(oc3 + 1) * 128],
                        out2_t[oc][:, sl],
                        start=(oc == 0), stop=(oc == CCW - 1))
                # t = conv3 + beta3 + x   (pre-ReLU, bf16)
                nc.vector.scalar_tensor_tensor(
                    out=t_t[oc3][:, sl], in0=ps, scalar=b3_t[oc3],
                    in1=xmid_t[oc3][:, sl], op0=ALU.add, op1=ALU.add)
                # fp8 relu copy (x RES_SCALE) for the inhibition matmul
                nc.scalar.activation(
                    out=resb_t[oc3 // 2][:, (oc3 % 2) * FR + ns * 512:
                                         (oc3 % 2) * FR + ns * 512 + 512],
                    in_=t_t[oc3][:, sl], func=AF.Relu, scale=RES_SCALE)
            for oc in range(CC3):
                ps4 = psum.tile([128, 512], F32, name="ps4", tag="mm512",
                                bufs=4)
                for cp in range(CC3 // 2):
                    lhsT = gdr_t[cp].rearrange(
                        "p (k o) -> p k o", k=2)[:, :, oc * 128:(oc + 1) * 128]
                    rhs = resb_t[cp].rearrange(
                        "p (k s) -> p k s", k=2)[:, :, sl]
                    nc.tensor.matmul(
                        ps4, lhsT, rhs, start=(cp == 0),
                        stop=(cp == CC3 // 2 - 1),
                        perf_mode=mybir.MatmulPerfMode.DoubleRow)
                den = div_pool.tile([128, 512], F32, name="den", tag="den")
                nc.scalar.activation(out=den, in_=ps4, func=AF.Copy,
                                     scale=DEN_SCALE, bias=1.0)
                rec = div_pool.tile([128, 512], F32, name="rec", tag="rec")
                nc.vector.reciprocal_approx_fast(out=rec, in_=den)
                # final = max(t, 0) * 1/(1+inh)
                nc.vector.scalar_tensor_tensor(
                    out=fin[:, oc * 512:(oc + 1) * 512],
                    in0=t_t[oc][:, sl], scalar=0.0, in1=rec,
                    op0=ALU.max, op1=ALU.mult)
            nc.sync.dma_start(
                out=ap["out"][:, ns],
                in_=fin.rearrange("p (c s) -> p c s", c=CC3))


def _prep_inputs(x, w1, g1, b1, m1, v1, lc_w, g2, b2, m2, v2,
                 w3, g3, b3, m3, v3, sigmas):
    """Host-side shard + layout prep. Returns per-core input maps."""
    f4 = np.float32
    x = np.asarray(x, f4)
    inv1 = (g1 / np.sqrt(v1 + EPS)).astype(f4)
    beta1 = (b1 - m1 * inv1).astype(f4)
    inv2 = (g2 / np.sqrt(v2 + EPS)).astype(f4)
    beta2 = (b2 - m2 * inv2).astype(f4)
    inv3 = (g3 / np.sqrt(v3 + EPS)).astype(f4)
    beta3 = (b3 - m3 * inv3).astype(f4)

    w1t = (np.asarray(w1, f4) * inv1[:, None]).T.reshape(CC1, 128, WID)
    w1t = np.ascontiguousarray(w1t).astype(NPBF16)
    w3t = (np.asarray(w3, f4) * inv3[:, None]).T.reshape(CCW, 128, COUT)
    w3t = np.ascontiguousarray(w3t).astype(NPBF16)

    # lc_w: (1,O,C,H,W,9) -> [h, w, p, (dk, ch, o)] with c = ch*128 + p.
    # inv2 is NOT folded (applied on-chip per o after the transpose);
    # values scaled by LCW_SCALE for fp8e3 range.
    lcw = np.asarray(lc_w[0], f4) * LCW_SCALE
    lcw = lcw.transpose(2, 3, 1, 4, 0)             # (H, W, C, K9, O)
    lcw = lcw.reshape(H, W, CCW, 128, 9, WID)      # (h, w, ch, p, dk, o)
    lcw = lcw.transpose(0, 1, 3, 4, 2, 5)          # (h, w, p, dk, ch, o)
    lcw = np.ascontiguousarray(lcw.reshape(H, W, 128, KC * WID)).astype(NPF8E3)

    # x in (c, h, w, n) layout
    xt = np.ascontiguousarray(x.transpose(1, 2, 3, 0))   # (C, H, W, N)
    xmidb = xt.astype(NPBF16)
    xhalo_src = np.zeros((CIN, H + 2, W, N), f4)
    xhalo_src[:, 1:H + 1] = xt * HALO_SCALE
    xhalob = xhalo_src.astype(NPF8E3)

    # gaussian distance template: gt[p, j] = (| |j-1024-p| - 512 |)^2
    jj = np.arange(2048)[None, :] - 1024 - np.arange(128)[:, None]
    gtm = ((np.abs(jj) - 512.0) ** 2).astype(f4).astype(NPBF16)

    sig = np.maximum(np.asarray(sigmas, f4), 0.5)
    sigs = (-1.0 / (2.0 * sig * sig)).reshape(CC3, 128, 1).astype(f4)

    com = {
        "ident": np.eye(128, dtype=NPBF16),
        "w1t": w1t, "w3t": w3t, "gt": gtm,
        "b1": beta1.reshape(CCW, 128, 1),
        "b2": beta2.reshape(CCW, 128, 1),
        "i2s": (inv2 / LCW_SCALE).astype(f4).reshape(CCW, 128, 1),
        "b3": beta3.reshape(CC3, 128, 1), "sigs": sigs,
    }
    in_maps = []
    for r in range(NCORES):
        r0 = r * RPC
        xm = np.ascontiguousarray(xmidb[:, r0:r0 + RPC]).reshape(
            CC1, 128, FR)
        xh = np.ascontiguousarray(
            xhalob[:, [r0, r0 + RPC + 1]]).reshape(CC1, 128, 2 * ROWF)
        lw = np.ascontiguousarray(lcw[r0:r0 + RPC]).reshape(
            NLOC, 128, KC * WID)
        if r == 0 or r == NCORES - 1:
            lw = lw.copy()
            if r == 0:           # row 0 locations: di=0 taps read row -1
                lw[0:W, :, 0:6 * WID] = 0
            if r == NCORES - 1:  # row 15 locations: di=2 taps read row 16
                lw[W:2 * W, :, 12 * WID:] = 0
        lwp = np.ascontiguousarray(
            lw.reshape(NPAIR, 2, 128, KC * WID).transpose(0, 2, 1, 3)
            .reshape(NPAIR, 128, 2 * KC * WID))
        in_maps.append(dict(com, xmid=xm, xhalo=xh, lcw=lwp))
    return in_maps


def _assemble(results):
    """results: per-core dicts with 'out' [128, 4, CC3, 512] bf16 (layout
    [p, ns, cchunk, s]; channel = cchunk*128+p, sample = ns*512+s over
    (hl, j, n)) -> (N, C, H, W) fp32."""
    full = np.empty((N, COUT, H, W), np.float32)
    for r, res in enumerate(results):
        o = res["out"].astype(np.float32)          # (128, 4, 8, 512)
        o = o.transpose(2, 0, 1, 3).reshape(COUT, FR)
        o = o.reshape(COUT, RPC, W, N).transpose(3, 0, 1, 2)
        full[:, :, r * RPC:(r + 1) * RPC, :] = o
    return full


_NC_CACHE = {}


def get_nc(ktimes: int = 1):
    if ktimes not in _NC_CACHE:
        _NC_CACHE[ktimes] = _build_nc(ktimes)
    return _NC_CACHE[ktimes]


def kernel(**inputs):
    nc = get_nc()
    in_maps = _prep_inputs(**inputs)
    res = run_bass_kernel_spmd(nc, in_maps, core_ids=list(range(NCORES)))
    return _assemble(res.results)
